# revision 14
# baseline (speedup 1.0000x reference)
"""DWA-CNN (DTW-aligned CNN) Trainium2 kernel, v2.

Problem: x[32,2048,128], w[3,128,8], b[8] -> out[32,2046,8]
out[b,p,f] = relu(b[f] + sum of dots along the DTW-optimal path between
window x[b,p:p+3,:] and filter w[:,:,f]).

v2 strategy (8 cores, data parallel over batch, 4 rows/core, TL=8192):
- Single bf16 matmul pass (x,w rounded to bf16; rel-err ~4e-3 << 2e-2).
- Stationary zero-padded to 96 cols so 4 position-blocks of 512 pack into
  one [96,512] PSUM tile -> one scalar evac per group (fp32->bf16).
- Scatter (24 rows -> 128 partitions) via DRAM bounce: 1 contiguous write
  per group + 3-dim linear reads (partition p = g*32 + f*4 + k makes the
  read AP linear; halo via a +512-offset strided read + tiny fix DMA).
- Whole DTW DP + backtrack in bf16 (2x DVE): masks/s-chain on GpSimd,
  spine + copy_predicated selects on DVE, evac/sqrt/relu on Scalar.
"""
import numpy as np

B, T, C, K, F = 32, 2048, 128, 3, 8
P = T - K + 1            # 2046
NCORES = 8
NB = B // NCORES         # 4 rows per core
TL = NB * T              # 8192 positions per core
FD = 512
NG = 4                   # groups of 4 blocks
JW = FD + 2              # 514: block + 2 halo cols
GW = K * JW + 2          # gm width (pad breaks false AP merges)
SCW = 3 * 32 * FD + 8    # scat dram words (pad 8)

_cache = {}


def _build_program():
    import concourse.tile as tile
    from concourse import bacc, mybir

    f32 = mybir.dt.float32
    bf16 = mybir.dt.bfloat16
    u16 = mybir.dt.uint16
    Alu = mybir.AluOpType
    Act = mybir.ActivationFunctionType

    nc = bacc.Bacc(
        "TRN2",
        target_bir_lowering=False,
        debug=False,
        enable_asserts=False,
        num_devices=NCORES,
    )

    xh = nc.dram_tensor("xh", [C, TL], bf16, kind="ExternalInput").ap()
    wp = nc.dram_tensor("wp", [C, 4 * 96], bf16, kind="ExternalInput").ap()
    nsr = nc.dram_tensor("nsr", [C, JW], bf16, kind="ExternalInput").ap()
    biasc = nc.dram_tensor("biasc", [C, 4], f32, kind="ExternalInput").ap()
    res = nc.dram_tensor("res", [C, FD], f32, kind="ExternalOutput").ap()

    from contextlib import ExitStack

    with tile.TileContext(nc) as tc, ExitStack() as ctx:
        const = ctx.enter_context(tc.tile_pool(name="const", bufs=1))
        xin = ctx.enter_context(tc.tile_pool(name="xin", bufs=1))
        psum = ctx.enter_context(tc.tile_pool(name="psum", bufs=1, space="PSUM"))
        stage = ctx.enter_context(tc.tile_pool(name="stage", bufs=2))
        arrs = ctx.enter_context(tc.tile_pool(name="arrs", bufs=1))
        work = ctx.enter_context(tc.tile_pool(name="work", bufs=1))
        dramp = ctx.enter_context(
            tc.tile_pool(name="dramp", bufs=1, space="DRAM"))

        wp_sb = const.tile([C, 4 * 96], bf16, tag="wp")
        nsr_sb = const.tile([C, JW], bf16, tag="nsr")
        bias_sb = const.tile([C, 4], f32, tag="bias")
        nc.gpsimd.dma_start(nsr_sb[:], nsr)

        # x: one tile per group; all a-halves (k=0/1 blocks) issued first
        xt = [xin.tile([C, 4 * FD], bf16, tag=f"x{g}", name=f"x{g}")
              for g in range(NG)]
        H = 2 * FD
        for g in range(NG):
            nc.sync.dma_start(xt[g][:, 0:H],
                              xh[:, g * 4 * FD:g * 4 * FD + H])
            if g == 0:
                nc.sync.dma_start(wp_sb[:], wp)
                nc.sync.dma_start(bias_sb[:], biasc)
        for g in range(NG):
            nc.sync.dma_start(xt[g][:, H:2 * H],
                              xh[:, g * 4 * FD + H:(g + 1) * 4 * FD])

        scratch = const.tile([C, 2], f32, tag="scratch")
        nc.scalar.activation(scratch[:, 0:1], bias_sb[:, 0:1],
                             Act.Sqrt)
        nc.scalar.activation(scratch[:, 1:2], bias_sb[:, 0:1],
                             Act.Relu)
        gm = arrs.tile([C, GW], bf16, tag="gm")
        scat = [dramp.tile([SCW], bf16, tag=f"scat{g}", name=f"scat{g}")
                for g in range(NG)]

        def scv(g):
            # [p(=f*4+k), j, e(514 over-read)] linear view of scat_g.
            # e=512,513 land on the next block's first cols: the correct
            # halo for k<3; for k=3 rows it's garbage that only feeds
            # windows at positions 2046/2047, which assembly discards.
            v = scat[g][0:3 * 32 * FD].rearrange(
                "(j p e) -> p j e", j=3, p=32)
            v.ap[-1] = [1, JW]
            return v

        def gmv(g):
            return gm[g * 32:(g + 1) * 32, 0:K * JW].rearrange(
                "p (j e) -> p j e", j=K)

        # k-outer matmul order: 4 stationary loads total (LDW overlap is
        # disabled in the toolchain, so per-block reloads cost ~130ns each)
        pss = [psum.tile([96, FD], f32, tag=f"ps{g}", name=f"ps{g}")
               for g in range(NG)]
        for k in range(4):
            for g in range(NG):
                nc.tensor.matmul(pss[g][:], wp_sb[:, k * 96:(k + 1) * 96],
                                 xt[g][:, k * FD:(k + 1) * FD],
                                 start=(k == 0), stop=(k == 3))
        stgs = []
        for g in range(NG):
            stg = stage.tile([96, FD], bf16, tag=f"stg{g}", name=f"stg{g}")
            nc.scalar.copy(stg[:], pss[g][:])
            stgs.append(stg)
            # bounce: contiguous mirror write
            nc.sync.dma_start(
                scat[g][0:96 * FD].rearrange("(p e) -> p e", e=FD), stg[:])
        for g in range(NG):
            # single over-reading main read per group (incl halo cols)
            reng = nc.scalar if g % 2 else nc.sync
            reng.dma_start(gmv(g), scv(g))
        V = nc.vector
        G = nc.gpsimd
        S = nc.scalar
        TT = V.tensor_tensor
        GT = G.tensor_tensor
        CP = V.copy_predicated

        Eg = arrs.tile([C, GW], bf16, tag="eg")
        Dj = arrs.tile([C, GW], bf16, tag="dj")

        for j in range(K):
            sl = slice(j * JW, j * JW + JW)
            TT(Eg[:, sl], gm[:, sl], nsr_sb[:], Alu.add)
            S.activation(Dj[:, sl], Eg[:, sl], Act.Sqrt,
                         bias=bias_sb[:, j:j + 1])

        # work mega-tiles; slots of FD bf16
        NW = 24
        W = work.tile([C, NW * FD], bf16, tag="W")
        M = work.tile([C, 8 * FD], u16, tag="M")
        (S_c12, S_c21, S_c13, S_c31, S_s10, S_s01, S_s20, S_X3,
         S_mbcA, S_mnA, S_c22, S_mbcB, S_mbcC, S_mnB, S_mnC, S_c23,
         S_c32, S_mbcD, S_X1, S_U, S_X2, S_XV1, S_X4, S_ACCM) = range(NW)
        (M_KA, M_KB, M_KC, M_KD, M_LA, M_LB, M_LC, M_LD) = range(8)

        def w1(s):
            return W[:, s * FD:(s + 1) * FD]

        def m1(s):
            return M[:, s * FD:(s + 1) * FD]

        def win2(ap2d, off_a, off_b, n=FD):
            v = ap2d[:, off_a:off_a + n].unsqueeze(1)
            v.ap[1] = [off_b - off_a, 2]
            return v

        def ww(sa, sb):
            return win2(W[:], sa * FD, sb * FD)

        def mw(sa, sb):
            return win2(M[:], sa * FD, sb * FD)

        def dwin(ia, ja, ib, jb):
            return win2(Dj[:], ja * JW + ia, jb * JW + ib)

        def gwin(ia, ja, ib, jb):
            return win2(gm[:], ja * JW + ia, jb * JW + ib)

        def dd(i, j):
            return Dj[:, j * JW + i:j * JW + i + FD]

        def gg(i, j):
            return gm[:, j * JW + i:j * JW + i + FD]

        # --- DP spine + s-chain ---
        # [c12|c21] = [D(0,1)|D(1,0)] + c11(x2)
        TT(ww(S_c12, S_c21), dwin(0, 1, 1, 0), dwin(0, 0, 0, 0), Alu.add)
        # [c13|c31] = [D(0,2)|D(2,0)] + [c12|c21]
        TT(ww(S_c13, S_c31), dwin(0, 2, 2, 0), ww(S_c12, S_c21), Alu.add)
        # s-chain on gpsimd (feeds selects late)
        TT(ww(S_s10, S_s01), gwin(1, 0, 0, 1), gwin(0, 0, 0, 0), Alu.add)
        TT(ww(S_s20, S_X3), gwin(2, 0, 0, 2), ww(S_s10, S_s01), Alu.add)

        TT(w1(S_mbcA), w1(S_c21), w1(S_c12), Alu.min)
        TT(w1(S_mnA), dd(0, 0), w1(S_mbcA), Alu.min)
        TT(w1(S_c22), dd(1, 1), w1(S_mnA), Alu.add)
        TT(m1(M_KA), dd(0, 0), w1(S_mbcA), Alu.is_le)
        # [mbcB|mbcC] = min([c22|c31],[c13|c22])
        TT(ww(S_mbcB, S_mbcC), ww(S_c22, S_c31), ww(S_c13, S_c22), Alu.min)
        # [mnB|mnC] = min([c12|c21], .)
        TT(ww(S_mnB, S_mnC), ww(S_c12, S_c21), ww(S_mbcB, S_mbcC), Alu.min)
        # [c23|c32] = [D(1,2)|D(2,1)] + [mnB|mnC]
        TT(ww(S_c23, S_c32), dwin(1, 2, 2, 1), ww(S_mnB, S_mnC), Alu.add)
        TT(w1(S_mbcD), w1(S_c32), w1(S_c23), Alu.min)

        # masks on DVE (u16 out needs DVE; 2x via 2-byte dtypes)
        TT(mw(M_LA, M_LB), ww(S_c21, S_c22), ww(S_c12, S_c13), Alu.is_le)
        TT(mw(M_KB, M_KC), ww(S_c12, S_c21), ww(S_mbcB, S_mbcC), Alu.is_le)
        TT(mw(M_LC, M_LD), ww(S_c31, S_c32), ww(S_c22, S_c23), Alu.is_le)
        TT(m1(M_KD), w1(S_c22), w1(S_mbcD), Alu.is_le)

        # --- select chain (DVE; copies via tensor_copy 4x) ---
        S.copy(w1(S_X1), w1(S_s01))
        CP(w1(S_X1), m1(M_LA), w1(S_s10))
        CP(w1(S_X1), m1(M_KA), gg(0, 0))
        TT(w1(S_U), gg(1, 1), w1(S_X1), Alu.add)
        S.copy(w1(S_X2), w1(S_U))
        # [X2|X3] where [LC|LB]: [s20|U]
        CP(ww(S_X2, S_X3), mw(M_LC, M_LB), ww(S_s20, S_U))
        # [X2|X3] where [KC|KB]: [s10|s01]
        CP(ww(S_X2, S_X3), mw(M_KC, M_KB), ww(S_s10, S_s01))
        # [XV1|X4] = [g(2,1)|g(1,2)] + [X2|X3]
        TT(ww(S_XV1, S_X4), gwin(2, 1, 1, 2), ww(S_X2, S_X3), Alu.add)
        CP(w1(S_X4), m1(M_LD), w1(S_XV1))
        CP(w1(S_X4), m1(M_KD), w1(S_U))
        resl = work.tile([C, FD], f32, tag="res")
        Hf = FD // 2
        for h in range(2):
            sl = slice(h * Hf, (h + 1) * Hf)
            TT(w1(S_ACCM)[:, sl],
               gg(2, 2)[:, sl], w1(S_X4)[:, sl], Alu.add)
            S.activation(resl[:, sl], w1(S_ACCM)[:, sl], Act.Relu,
                         bias=bias_sb[:, 3:4], scale=-0.5)
            eng = nc.sync if h == 0 else nc.scalar
            eng.dma_start(res[:, sl], resl[:, sl])

    nc.compile()
    return nc


def _host_prep(x, w, b):
    import ml_dtypes

    x = np.ascontiguousarray(np.asarray(x, np.float32))
    w = np.asarray(w, np.float32)
    b = np.asarray(b, np.float32)

    # stationary: wp[:, k*96 + (j*32 + f*4 + kslot)] = -2w[j,:,f] iff kslot==k
    wp = np.zeros((C, 4 * 96), np.float32)
    for k in range(4):
        for j in range(K):
            for f in range(F):
                wp[:, k * 96 + j * 32 + f * 4 + k] = -2.0 * w[j, :, f]
    wp = wp.astype(ml_dtypes.bfloat16)

    nW = (w ** 2).sum(1)                                  # [K, F]
    biasc = np.zeros((C, 4), np.float32)
    for p in range(C):
        f = (p % 32) // 4
        for j in range(K):
            biasc[p, j] = nW[j, f]
        biasc[p, 3] = b[f]

    in_maps = []
    for r in range(NCORES):
        x4 = x[r * NB:(r + 1) * NB]                       # [NB,T,C]
        flat = x4.reshape(TL, C)
        xT = np.ascontiguousarray(flat.T)                 # [C, TL] fp32
        xhh = xT.astype(ml_dtypes.bfloat16)
        nS = np.einsum("tc,tc->t", flat, flat).astype(np.float32)
        nsp = np.ones((C, JW), np.float32)
        for p in range(C):
            g = p // 32
            k = p % 4
            t0 = (4 * g + k) * FD
            hi = min(TL, t0 + JW)
            nsp[p, 0:hi - t0] = nS[t0:hi]
        in_maps.append({
            "xh": xhh, "wp": wp, "nsr": nsp.astype(ml_dtypes.bfloat16),
            "biasc": biasc,
        })
    return in_maps


def _assemble(results):
    out = np.empty((B, P, F), np.float32)
    for r in range(NCORES):
        resr = results[r]["res"]                          # [128, 512] f32
        arr = resr.reshape(4, 8, 4, FD)                   # [g, f, k, e]
        # out[r*NB+g, k*512+e, f] = arr[g, f, k, e]
        series = arr.transpose(0, 2, 3, 1).reshape(4, T, F)  # [g, pos, f]
        out[r * NB:(r + 1) * NB] = series[:, :P, :]
    return out


def kernel(x, w, b):
    from concourse.bass_utils import run_bass_kernel_spmd

    if "nc" not in _cache:
        _cache["nc"] = _build_program()
    nc = _cache["nc"]
    in_maps = _host_prep(x, w, b)
    out = run_bass_kernel_spmd(nc, in_maps, core_ids=list(range(NCORES)))
    return _assemble(out.results)


if __name__ == "__main__":
    rng = np.random.default_rng(0)
    x = rng.standard_normal((B, T, C), dtype=np.float32)
    w = (rng.standard_normal((K, C, F)) * 0.1).astype(np.float32)
    b = np.zeros((F,), np.float32)
    o = kernel(x, w, b)
    print("kernel ran, out shape", o.shape, float(np.abs(o).sum()))
